# revision 1
# baseline (speedup 1.0000x reference)
"""Trainium2 Bass kernel for a single RoBERTa encoder layer.

Problem: B=8, S=512, H=1024, 16 heads (d=64), FF=4096, fp32, eval mode.

Strategy: pure data-parallel over batch — one batch element per NeuronCore
(8 cores), full weights replicated. Inside a core everything flows in a
"feature-on-partitions, tokens-on-free" transposed layout so that every
projection / FFN matmul takes weights in natural layout as the stationary
operand and activations as the moving operand with N=512 — which at
float32r (FP22 read of fp32 bits) runs at full PE rate (1 cycle/row).

Per-core pipeline:
  X [512,1024] --PE transpose--> XT (8 x [128,512] f32r)
  QT/KT = W.T @ XT (+bias via ACT Identity)      [h' on partitions]
  V'' normal layout with per-head [V_h | ones64] interleaved -> ctx matmul
  per head h: scoresT[kpos,q] = KT_h.T(slice) @ QT_h ; exp via ACT
              ctx' [128,512] = [V_h|ones].T @ expT  (rows 64:128 = sumexp)
              rinv = reciprocal(rows 64:128); ctxTh = psum[0:64] * rinv
  pack head pairs via selection matmuls ([I|0],[0|I]) -> ctxT pairs
  OT = Wo.T @ ctxT ; zT = OT + bo + XT (residual)
  LN1 in transposed layout: ones-matmul stats + K=1 broadcast matmuls
  interT = gelu(Wi.T @ attn_outT + bi) via ACT Gelu  [f on partitions]
  out2T = Wo2.T @ interT ; z2T = out2T + bo2 + attn_outT
  PE-transpose z2T -> z2 normal ; LN2 via bn_stats ; contiguous DMA out.
"""
import numpy as np

import concourse.bass as bass
import concourse.mybir as mybir
import concourse.tile as tile
from concourse import bacc
from concourse import bass_utils
from concourse.masks import make_identity

F32 = mybir.dt.float32
F32R = mybir.dt.float32r
AF = mybir.ActivationFunctionType
ALU = mybir.AluOpType

B, S, H, NH, HD, FF = 8, 512, 1024, 16, 64, 4096
KT_H = H // 128   # 8 k tiles over hidden
FT = FF // 128    # 32 f tiles over intermediate
ST = S // 128     # 4 token tiles
EPS = 1e-5

_CACHE = {}
DEBUG = False


def _build():
    nc = bacc.Bacc("TRN2", target_bir_lowering=False, debug=False,
                   enable_asserts=True, num_devices=B)

    def din(name, shape, dt=F32R):
        return nc.dram_tensor(name, shape, dt, kind="ExternalInput").ap()

    x = din("x", [S, H], F32)
    maskt = din("maskt", [128, ST], F32)        # per-core mask, host-transposed
    wq, wk, wv = din("wq", [H, H]), din("wk", [H, H]), din("wv", [H, H])
    wo, wi, wo2 = din("wo", [H, H]), din("wi", [H, FF]), din("wo2", [FF, H])
    bv = din("bv", [H], F32)
    # host-packed per-partition vectors: bq bk bo bo2 g1 b1 g2 b2 (8 cols each)
    # then bi (32 cols) -> [128, 96]
    biast = din("biast", [128, 96], F32)
    ones_col = din("ones_col", [128, 1])        # f32r ones
    ones_row = din("ones_row", [1, 128])        # f32r ones
    ones64 = din("ones64", [128, 64])           # f32r ones block for V''
    sel_a = din("sel_a", [128, 128])            # [[I64|0];[0|0]]
    sel_b = din("sel_b", [128, 128])            # [[0|I64];[0|0]]
    zeros64 = din("zeros64", [128, 64])         # f32r zeros
    out = nc.dram_tensor("out", [S, H], F32, kind="ExternalOutput").ap()
    dbg = {}
    if DEBUG:
        for nm, shp in [("d_xt0", [128, 512]), ("d_qt0", [128, 512]),
                        ("d_kt0", [128, 512]), ("d_vv0", [128, 2048]),
                        ("d_exp00", [128, 512]), ("d_ctx0", [128, 512]),
                        ("d_zt0", [128, 512]), ("d_aot0", [128, 512]),
                        ("d_int0", [128, 512]), ("d_ctxraw0", [128, 512]),
                        ("d_rinv0", [64, 512])]:
            dbg[nm] = nc.dram_tensor(nm, shp, F32, kind="ExternalOutput").ap()

    from contextlib import ExitStack
    es = ExitStack()
    with tile.TileContext(nc) as tc, es:
        # ---- long-lived pools (manually scoped via ExitStack.close) ----
        # LEFT stack: cst > p_xt > {phase transients} ... then p_int > {ffn}
        # RIGHT stack: p_aot > p_zt > p_pair (closed in reverse phase order)
        cst = es.enter_context(tc.tile_pool(name="cst", bufs=1))
        es_xt = ExitStack()
        p_xt = es_xt.enter_context(tc.tile_pool(name="p_xt", bufs=8))
        es_qk = ExitStack()
        es_pair = ExitStack()
        es_zt = ExitStack()
        es_aot = ExitStack()
        es_int = ExitStack()

        # ---- identity first (gates phase-1 transposes) ----
        ident = cst.tile([128, 128], F32, tag="ident")
        make_identity(nc, ident)

        # ---- phase 1: X -> XT (PE transposes) ----
        xt = [p_xt.tile([128, S], F32R, tag="xt", name=f"xt{i}") for i in range(KT_H)]
        with tc.tile_pool(name="p_x", bufs=2) as p_x, \
             tc.tile_pool(name="ps_tp", bufs=3, space="PSUM") as ps_tp:
            for i in range(ST):
                x_sb = p_x.tile([128, H], F32, tag="x")
                nc.sync.dma_start(out=x_sb, in_=x[128 * i:128 * (i + 1), :])
                for j in range(KT_H):
                    pt = ps_tp.tile([128, 128], F32, tag="tp")
                    nc.tensor.transpose(pt, x_sb[:, 128 * j:128 * (j + 1)], ident)
                    nc.vector.tensor_copy(out=xt[j][:, 128 * i:128 * (i + 1)], in_=pt)

        if DEBUG:
            nc.sync.dma_start(out=dbg["d_xt0"], in_=xt[0].bitcast(F32))

        # ---- constants / biases (emitted after x so x wins DMA priority) ----
        t_biast = cst.tile([128, 96], F32, tag="t_biast")
        nc.sync.dma_start(out=t_biast, in_=biast)
        t_bq, t_bk = t_biast[:, 0:8], t_biast[:, 8:16]
        t_bo, t_bo2 = t_biast[:, 16:24], t_biast[:, 24:32]
        t_g1, t_b1 = t_biast[:, 32:40], t_biast[:, 40:48]
        t_g2, t_b2 = t_biast[:, 48:56], t_biast[:, 56:64]
        t_bi = t_biast[:, 64:96]
        t_mask = cst.tile([128, ST], F32, tag="t_mask")
        nc.sync.dma_start(out=t_mask, in_=maskt)
        t_ones_col = cst.tile([128, 1], F32R, tag="t_ones_col")
        nc.sync.dma_start(out=t_ones_col, in_=ones_col)
        t_ones_row = cst.tile([1, 128], F32R, tag="t_ones_row")
        nc.sync.dma_start(out=t_ones_row, in_=ones_row)
        t_ones64 = cst.tile([128, 64], F32R, tag="t_ones64")
        nc.sync.dma_start(out=t_ones64, in_=ones64)
        t_sel_a = cst.tile([128, 128], F32R, tag="t_sel_a")
        nc.sync.dma_start(out=t_sel_a, in_=sel_a)
        t_sel_b = cst.tile([128, 128], F32R, tag="t_sel_b")
        nc.sync.dma_start(out=t_sel_b, in_=sel_b)
        t_zeros64 = cst.tile([128, 64], F32R, tag="t_zeros64")
        nc.sync.dma_start(out=t_zeros64, in_=zeros64)
        t_eps1 = cst.tile([1, 1], F32, tag="t_eps1")
        nc.vector.memset(t_eps1, EPS)
        t_eps128 = cst.tile([128, 1], F32, tag="t_eps128")
        nc.vector.memset(t_eps128, EPS)

        # ---- phase 2: QKV projections ----
        p_aot = es_aot.enter_context(tc.tile_pool(name="p_aot", bufs=8, side="right"))
        p_zt = es_zt.enter_context(tc.tile_pool(name="p_zt", bufs=8, side="right"))
        p_pair = es_pair.enter_context(tc.tile_pool(name="p_pair", bufs=8, side="right"))
        p_qt = es_qk.enter_context(tc.tile_pool(name="p_qt", bufs=16))
        p_kt = es_qk.enter_context(tc.tile_pool(name="p_kt", bufs=8))
        p_v = es_qk.enter_context(tc.tile_pool(name="p_v", bufs=4))
        qta = [p_qt.tile([128, S], F32R, tag="qt", name=f"qta{i}") for i in range(KT_H)]
        qtb = [p_qt.tile([128, S], F32R, tag="qt", name=f"qtb{i}") for i in range(KT_H)]
        kt = [p_kt.tile([128, S], F32R, tag="kt", name=f"ktt{i}") for i in range(KT_H)]
        for m in range(KT_H):  # zero the unused halves once
            nc.vector.tensor_copy(
                out=qta[m][64:128, :].rearrange("p (o c) -> p o c", c=64),
                in_=t_zeros64[64:128, :].rearrange(
                    "p (o c) -> p o c", o=1).broadcast_to([64, 8, 64]))
            nc.vector.tensor_copy(
                out=qtb[m][0:64, :].rearrange("p (o c) -> p o c", c=64),
                in_=t_zeros64[0:64, :].rearrange(
                    "p (o c) -> p o c", o=1).broadcast_to([64, 8, 64]))
        # V'' tiles: per s-tile [128, 2048]: head h at cols 128h..128h+64 = V,
        # cols 128h+64..128h+128 = ones
        vv = [p_v.tile([128, 16 * 128], F32R, tag="vv", name=f"vv{i}") for i in range(ST)]
        for s in range(ST):
            nc.vector.tensor_copy(
                out=vv[s].rearrange("p (h c) -> p h c", c=128)[:, :, 64:128],
                in_=t_ones64.rearrange("p (o c) -> p o c", o=1).broadcast_to(
                    [128, 16, 64]))

        with tc.tile_pool(name="p_w", bufs=12) as p_w, \
             tc.tile_pool(name="p_bv", bufs=1) as p_bv, \
             tc.tile_pool(name="ps_qkv", bufs=3, space="PSUM") as ps_qkv:
            # Q and K: transposed outputs, weights stationary
            t_bv_rep = p_bv.tile([128, H], F32, tag="t_bv_rep")
            nc.sync.dma_start(
                out=t_bv_rep,
                in_=bv.rearrange("(o n) -> o n", o=1).partition_broadcast(128))
            for w_dram, bias_t, which in ((wq, t_bq, "q"), (wk, t_bk, "k")):
                wt = []
                for k in range(KT_H):
                    w_sb = p_w.tile([128, H], F32R, tag="w4k")
                    nc.sync.dma_start(out=w_sb, in_=w_dram[128 * k:128 * (k + 1), :])
                    wt.append(w_sb)
                for m in range(KT_H):
                    ps = ps_qkv.tile([128, S], F32, tag="pq")
                    for k in range(KT_H):
                        nc.tensor.matmul(ps, wt[k][:, 128 * m:128 * (m + 1)], xt[k],
                                         start=(k == 0), stop=(k == KT_H - 1))
                    if which == "q":
                        nc.scalar.activation(out=qta[m][0:64, :], in_=ps[0:64, :],
                                             func=AF.Identity,
                                             bias=bias_t[0:64, m:m + 1], scale=1.0)
                        nc.scalar.activation(out=qtb[m][64:128, :], in_=ps[64:128, :],
                                             func=AF.Identity,
                                             bias=bias_t[64:128, m:m + 1], scale=1.0)
                    else:
                        nc.scalar.activation(out=kt[m], in_=ps, func=AF.Identity,
                                             bias=bias_t[:, m:m + 1], scale=1.0)
            # V: normal layout, X stationary
            wvt = []
            for k in range(KT_H):
                w_sb = p_w.tile([128, H], F32R, tag="w4k")
                nc.sync.dma_start(out=w_sb, in_=wv[128 * k:128 * (k + 1), :])
                wvt.append(w_sb)
            for s in range(ST):
                for n in range(2):
                    ps = ps_qkv.tile([128, 512], F32, tag="pq")
                    for k in range(KT_H):
                        nc.tensor.matmul(
                            ps, xt[k][:, 128 * s:128 * (s + 1)],
                            wvt[k][:, 512 * n:512 * (n + 1)],
                            start=(k == 0), stop=(k == KT_H - 1))
                    nc.vector.tensor_tensor(
                        out=vv[s].rearrange("p (h c) -> p h c", c=128)[:, 8 * n:8 * n + 8, 0:64],
                        in0=ps.rearrange("p (h c) -> p h c", c=64),
                        in1=t_bv_rep[:, 512 * n:512 * (n + 1)].rearrange(
                            "p (h c) -> p h c", c=64),
                        op=ALU.add)

        if DEBUG:
            nc.sync.dma_start(out=dbg["d_qt0"], in_=qta[0].bitcast(F32))
            nc.sync.dma_start(out=dbg["d_kt0"], in_=kt[0].bitcast(F32))
            nc.sync.dma_start(out=dbg["d_vv0"], in_=vv[0].bitcast(F32))

        # ---- phase 3: attention per head; pack pairs ----
        pair = [p_pair.tile([128, S], F32R, tag="pair", name=f"pair{i}") for i in range(KT_H)]
        with tc.tile_pool(name="p_expt", bufs=10) as p_expt, \
             tc.tile_pool(name="p_ctxh", bufs=1) as p_ctxh, \
             tc.tile_pool(name="p_rinv", bufs=4) as p_rinv, \
             tc.tile_pool(name="ps_sc", bufs=4, space="PSUM") as ps_sc, \
             tc.tile_pool(name="ps_ctx", bufs=2, space="PSUM") as ps_ctx, \
             tc.tile_pool(name="ps_shift", bufs=2, space="PSUM") as ps_shift:
            ctxh_t = [p_ctxh.tile([128, S], F32R, tag=f"ctxh{i}", name=f"ctxh{i}")
                      for i in range(4)]
            for i in range(4):  # zero rows 64:128 once (shift matmul safety)
                nc.vector.tensor_copy(
                    out=ctxh_t[i][64:128, :].rearrange("p (o c) -> p o c", c=64),
                    in_=t_zeros64[64:128, :].rearrange(
                        "p (o c) -> p o c", o=1).broadcast_to([64, 8, 64]))
            for t in range(KT_H):  # pair index
                expts = {}
                for h in (2 * t, 2 * t + 1):
                    mt = h // 2
                    qthalf = qta[mt] if h % 2 == 0 else qtb[mt]
                    expt = []
                    for kt_i in range(ST):
                        ps_s = ps_sc.tile([128, S], F32, tag="sc")
                        nc.tensor.matmul(
                            ps_s,
                            kt[mt][:, 128 * kt_i:128 * (kt_i + 1)],
                            qthalf,
                            start=True, stop=True)
                        e = p_expt.tile([128, S], F32R, tag="expt")
                        nc.scalar.activation(out=e, in_=ps_s, func=AF.Exp,
                                             bias=t_mask[:, kt_i:kt_i + 1],
                                             scale=1.0 / np.sqrt(HD))
                        expt.append(e)
                        if DEBUG and h == 0 and kt_i == 0:
                            nc.sync.dma_start(out=dbg["d_exp00"], in_=e.bitcast(F32))
                    expts[h] = expt
                ctxh = {}
                for h in (2 * t, 2 * t + 1):
                    expt = expts[h]
                    ps_c = ps_ctx.tile([128, S], F32, tag="ctx")
                    for kt_i in range(ST):
                        nc.tensor.matmul(
                            ps_c, vv[kt_i][:, 128 * h:128 * (h + 1)], expt[kt_i],
                            start=(kt_i == 0), stop=(kt_i == ST - 1))
                    if DEBUG and h == 0:
                        craw = p_ctxh.tile([128, S], F32, tag="craw")
                        nc.vector.tensor_copy(out=craw, in_=ps_c)
                        nc.sync.dma_start(out=dbg["d_ctxraw0"], in_=craw)
                    sums_sb = p_rinv.tile([64, S], F32, tag="sums_sb")
                    nc.vector.tensor_copy(out=sums_sb, in_=ps_c[64:128, :])
                    rinv = p_rinv.tile([64, S], F32, tag="rinv")
                    nc.vector.reciprocal_approx_fast(out=rinv, in_=sums_sb)
                    if DEBUG and h == 0:
                        nc.sync.dma_start(out=dbg["d_rinv0"], in_=rinv)
                    ch = ctxh_t[h % 4]
                    nc.vector.tensor_tensor(out=ch[0:64, :], in0=ps_c[0:64, :],
                                            in1=rinv, op=ALU.mult)
                    ctxh[h] = ch
                ps_p = ps_shift.tile([128, S], F32, tag="shift")
                nc.tensor.matmul(ps_p, t_sel_a, ctxh[2 * t], start=True, stop=False)
                nc.tensor.matmul(ps_p, t_sel_b, ctxh[2 * t + 1], start=False, stop=True)
                nc.vector.tensor_copy(out=pair[t], in_=ps_p)
                if DEBUG and t == 0:
                    nc.sync.dma_start(out=dbg["d_ctx0"], in_=pair[0].bitcast(F32))
        es_qk.close()

        # ---- phase 4: attention output proj + residual (transposed) ----
        zt = [p_zt.tile([128, S], F32R, tag="zt", name=f"zt{i}") for i in range(KT_H)]
        with tc.tile_pool(name="p_w2", bufs=9) as p_w2, \
             tc.tile_pool(name="p_tsum", bufs=2) as p_tsum, \
             tc.tile_pool(name="ps_wo", bufs=3, space="PSUM") as ps_wo:
            wot = []
            for k in range(KT_H):
                w_sb = p_w2.tile([128, H], F32R, tag="wo4k")
                nc.sync.dma_start(out=w_sb, in_=wo[128 * k:128 * (k + 1), :])
                wot.append(w_sb)
            for m in range(KT_H):
                ps = ps_wo.tile([128, S], F32, tag="wo")
                for k in range(KT_H):
                    nc.tensor.matmul(ps, wot[k][:, 128 * m:128 * (m + 1)], pair[k],
                                     start=(k == 0), stop=(k == KT_H - 1))
                # zT = OT + bo + XT
                tsum = p_tsum.tile([128, S], F32, tag="tsum")
                nc.vector.tensor_tensor(out=tsum, in0=ps, in1=xt[m], op=ALU.add)
                nc.scalar.activation(out=zt[m], in_=tsum, func=AF.Identity,
                                     bias=t_bo[:, m:m + 1], scale=1.0)
        if DEBUG:
            nc.sync.dma_start(out=dbg["d_zt0"], in_=zt[0].bitcast(F32))
        es_pair.close()
        es_xt.close()

        # ---- phase 5: LN1 in transposed layout ----
        aot = [p_aot.tile([128, S], F32R, tag="aot", name=f"aot{i}") for i in range(KT_H)]
        with tc.tile_pool(name="p_sq", bufs=2) as p_sq, \
             tc.tile_pool(name="p_stat", bufs=1) as p_stat, \
             tc.tile_pool(name="ps_stat", bufs=2, space="PSUM") as ps_stat, \
             tc.tile_pool(name="ps_rep", bufs=2, space="PSUM") as ps_rep:
            ps_sum = ps_stat.tile([1, S], F32, tag="lnsum")
            ps_sumsq = ps_stat.tile([1, S], F32, tag="lnsum")
            for m in range(KT_H):
                sq = p_sq.tile([128, S], F32R, tag="sq")
                nc.vector.tensor_tensor(out=sq, in0=zt[m], in1=zt[m], op=ALU.mult)
                nc.tensor.matmul(ps_sum, t_ones_col, zt[m],
                                 start=(m == 0), stop=(m == KT_H - 1))
                nc.tensor.matmul(ps_sumsq, t_ones_col, sq,
                                 start=(m == 0), stop=(m == KT_H - 1))
            mu = p_stat.tile([1, S], F32R, tag="mu")
            nc.vector.tensor_scalar(out=mu, in0=ps_sum, scalar1=1.0 / H,
                                    scalar2=None, op0=ALU.mult)
            ex2 = p_stat.tile([1, S], F32, tag="ex2")
            nc.vector.tensor_scalar(out=ex2, in0=ps_sumsq, scalar1=1.0 / H,
                                    scalar2=None, op0=ALU.mult)
            mu2 = p_stat.tile([1, S], F32, tag="mu2")
            nc.vector.tensor_tensor(out=mu2, in0=mu, in1=mu, op=ALU.mult)
            var = p_stat.tile([1, S], F32, tag="var")
            nc.vector.tensor_tensor(out=var, in0=ex2, in1=mu2, op=ALU.subtract)
            sd = p_stat.tile([1, S], F32, tag="sd")
            nc.scalar.activation(out=sd, in_=var, func=AF.Sqrt, bias=t_eps1, scale=1.0)
            rstd_f = p_stat.tile([1, S], F32, tag="rstd_f")
            nc.vector.reciprocal_approx_fast(out=rstd_f, in_=sd)
            rstd = p_stat.tile([1, S], F32R, tag="rstd")
            nc.scalar.activation(out=rstd, in_=rstd_f, func=AF.Identity)
            ps_mu = ps_rep.tile([128, S], F32, tag="murep")
            nc.tensor.matmul(ps_mu, t_ones_row, mu, start=True, stop=True)
            ps_rstd = ps_rep.tile([128, S], F32, tag="murep")
            nc.tensor.matmul(ps_rstd, t_ones_row, rstd, start=True, stop=True)
            for m in range(KT_H):
                t1 = p_sq.tile([128, S], F32, tag="t1")
                nc.vector.tensor_tensor(out=t1, in0=zt[m], in1=ps_mu, op=ALU.subtract)
                t2 = p_sq.tile([128, S], F32, tag="t2")
                nc.vector.tensor_tensor(out=t2, in0=t1, in1=ps_rstd, op=ALU.mult)
                nc.scalar.activation(out=aot[m], in_=t2, func=AF.Identity,
                                     bias=t_b1[:, m:m + 1], scale=t_g1[:, m:m + 1])
        es_zt.close()

        if DEBUG:
            nc.sync.dma_start(out=dbg["d_aot0"], in_=aot[0].bitcast(F32))

        # ---- phase 6: FFN1 (gelu) ----
        p_int = es_int.enter_context(tc.tile_pool(name="p_int", bufs=32))
        intert = [p_int.tile([128, S], F32R, tag="intert", name=f"intert{i}") for i in range(FT)]
        with tc.tile_pool(name="p_wi", bufs=16) as p_wi, \
             tc.tile_pool(name="ps_f1", bufs=4, space="PSUM") as ps_f1:
            for fb in range(FT // 4):  # blocks of 4 f-tiles
                wi_chunks = []
                for k in range(KT_H):
                    c = p_wi.tile([128, 512], F32R, tag="wi")
                    nc.sync.dma_start(
                        out=c, in_=wi[128 * k:128 * (k + 1), 512 * fb:512 * (fb + 1)])
                    wi_chunks.append(c)
                for fi in range(4):
                    f = 4 * fb + fi
                    ps = ps_f1.tile([128, S], F32, tag="f1")
                    for k in range(KT_H):
                        nc.tensor.matmul(
                            ps, wi_chunks[k][:, 128 * fi:128 * (fi + 1)], aot[k],
                            start=(k == 0), stop=(k == KT_H - 1))
                    nc.scalar.activation(out=intert[f], in_=ps, func=AF.Gelu,
                                         bias=t_bi[:, f:f + 1], scale=1.0)

        if DEBUG:
            nc.sync.dma_start(out=dbg["d_int0"], in_=intert[0].bitcast(F32))

        # ---- phase 7: FFN2 + residual + LN2 (transpose back) + out ----
        with tc.tile_pool(name="p_wo2", bufs=6) as p_wo2, \
             tc.tile_pool(name="p_z2t", bufs=8) as p_z2t, \
             tc.tile_pool(name="p_tsum2", bufs=3) as p_tsum2, \
             tc.tile_pool(name="p_res", bufs=4) as p_res:
            with tc.tile_pool(name="ps_f2", bufs=8, space="PSUM") as ps_f2:
                ps_o2 = [ps_f2.tile([128, S], F32, tag="f2", name=f"ps_o2_{i}") for i in range(KT_H)]
                for f in range(FT):
                    w_sb = p_wo2.tile([128, H], F32R, tag="wo2")
                    nc.sync.dma_start(out=w_sb, in_=wo2[128 * f:128 * (f + 1), :])
                    for m in range(KT_H):
                        nc.tensor.matmul(ps_o2[m], w_sb[:, 128 * m:128 * (m + 1)],
                                         intert[f], start=(f == 0), stop=(f == FT - 1))
                    z2t = []
                for m in range(KT_H):
                    tsum = p_tsum2.tile([128, S], F32, tag="tsum2")
                    nc.vector.tensor_tensor(out=tsum, in0=ps_o2[m], in1=aot[m], op=ALU.add)
                    zz = p_z2t.tile([128, S], F32R, tag="z2t")
                    nc.scalar.activation(out=zz, in_=tsum, func=AF.Identity,
                                         bias=t_bo2[:, m:m + 1], scale=1.0)
                    z2t.append(zz)
            # LN2 in transposed layout, then transpose result + store
            with tc.tile_pool(name="p_sq2", bufs=2) as p_sq2, \
                 tc.tile_pool(name="p_stat2", bufs=1) as p_stat2, \
                 tc.tile_pool(name="p_y", bufs=8) as p_y, \
                 tc.tile_pool(name="ps_stat2", bufs=2, space="PSUM") as ps_stat2, \
                 tc.tile_pool(name="ps_rep2", bufs=2, space="PSUM") as ps_rep2, \
                 tc.tile_pool(name="ps_tp2", bufs=3, space="PSUM") as ps_tp2:
                ps2_sum = ps_stat2.tile([1, S], F32, tag="ln2sum")
                ps2_sumsq = ps_stat2.tile([1, S], F32, tag="ln2sum")
                for m in range(KT_H):
                    sq2 = p_sq2.tile([128, S], F32R, tag="sq2")
                    nc.vector.tensor_tensor(out=sq2, in0=z2t[m], in1=z2t[m],
                                            op=ALU.mult)
                    nc.tensor.matmul(ps2_sum, t_ones_col, z2t[m],
                                     start=(m == 0), stop=(m == KT_H - 1))
                    nc.tensor.matmul(ps2_sumsq, t_ones_col, sq2,
                                     start=(m == 0), stop=(m == KT_H - 1))
                mu2t = p_stat2.tile([1, S], F32R, tag="mu2t")
                nc.vector.tensor_scalar(out=mu2t, in0=ps2_sum, scalar1=1.0 / H,
                                        scalar2=None, op0=ALU.mult)
                ex2b = p_stat2.tile([1, S], F32, tag="ex2b")
                nc.vector.tensor_scalar(out=ex2b, in0=ps2_sumsq, scalar1=1.0 / H,
                                        scalar2=None, op0=ALU.mult)
                mu2sq = p_stat2.tile([1, S], F32, tag="mu2sq")
                nc.vector.tensor_tensor(out=mu2sq, in0=mu2t, in1=mu2t, op=ALU.mult)
                var2 = p_stat2.tile([1, S], F32, tag="var2")
                nc.vector.tensor_tensor(out=var2, in0=ex2b, in1=mu2sq,
                                        op=ALU.subtract)
                sd2 = p_stat2.tile([1, S], F32, tag="sd2")
                nc.scalar.activation(out=sd2, in_=var2, func=AF.Sqrt,
                                     bias=t_eps1, scale=1.0)
                rstd2f = p_stat2.tile([1, S], F32, tag="rstd2f")
                nc.vector.reciprocal_approx_fast(out=rstd2f, in_=sd2)
                rstd2 = p_stat2.tile([1, S], F32R, tag="rstd2")
                nc.scalar.activation(out=rstd2, in_=rstd2f, func=AF.Identity)
                ps_mu2 = ps_rep2.tile([128, S], F32, tag="mu2rep")
                nc.tensor.matmul(ps_mu2, t_ones_row, mu2t, start=True, stop=True)
                ps_rstd2 = ps_rep2.tile([128, S], F32, tag="mu2rep")
                nc.tensor.matmul(ps_rstd2, t_ones_row, rstd2, start=True, stop=True)
                stg = [p_res.tile([128, H], F32, tag="stg", name=f"stg{i}")
                       for i in range(ST)]
                for m in range(KT_H):
                    u1 = p_sq2.tile([128, S], F32, tag="u1")
                    nc.vector.tensor_tensor(out=u1, in0=z2t[m], in1=ps_mu2,
                                            op=ALU.subtract)
                    u2 = p_sq2.tile([128, S], F32, tag="u2")
                    nc.vector.tensor_tensor(out=u2, in0=u1, in1=ps_rstd2,
                                            op=ALU.mult)
                    y = p_y.tile([128, S], F32, tag="y", name=f"y{m}")
                    nc.scalar.activation(out=y, in_=u2, func=AF.Identity,
                                         bias=t_b2[:, m:m + 1],
                                         scale=t_g2[:, m:m + 1])
                    for s_i in range(ST):
                        pt = ps_tp2.tile([128, 128], F32, tag="tp2")
                        nc.tensor.transpose(
                            pt, y[:, 128 * s_i:128 * (s_i + 1)], ident)
                        nc.vector.tensor_copy(
                            out=stg[s_i][:, 128 * m:128 * (m + 1)], in_=pt)
                for s_i in range(ST):
                    nc.sync.dma_start(out=out[128 * s_i:128 * (s_i + 1), :],
                                      in_=stg[s_i])
        es_int.close()
        es_aot.close()

    nc.compile()
    return nc


def _get_nc():
    if "nc" not in _CACHE:
        _CACHE["nc"] = _build()
    return _CACHE["nc"]


def _perpart(v):
    # [n*128] -> [128, n] with vT[p, t] = v[t*128 + p]
    v = np.asarray(v, np.float32)
    return np.ascontiguousarray(v.reshape(-1, 128).T)


def _shared_inputs(inp):
    f = np.float32
    biast = np.concatenate(
        [_perpart(inp["bq"]), _perpart(inp["bk"]), _perpart(inp["bo"]),
         _perpart(inp["bo2"]), _perpart(inp["ln1_g"]), _perpart(inp["ln1_b"]),
         _perpart(inp["ln2_g"]), _perpart(inp["ln2_b"]), _perpart(inp["bi"])],
        axis=1)
    return {
        "wq": np.ascontiguousarray(inp["wq"], f),
        "wk": np.ascontiguousarray(inp["wk"], f),
        "wv": np.ascontiguousarray(inp["wv"], f),
        "wo": np.ascontiguousarray(inp["wo"], f),
        "wi": np.ascontiguousarray(inp["wi"], f),
        "wo2": np.ascontiguousarray(inp["wo2"], f),
        "bv": np.ascontiguousarray(inp["bv"], f),
        "biast": biast,
        "ones_col": np.ones((128, 1), f),
        "ones_row": np.ones((1, 128), f),
        "ones64": np.ones((128, 64), f),
        "sel_a": np.concatenate(
            [np.concatenate([np.eye(64, dtype=f), np.zeros((64, 64), f)], axis=1),
             np.zeros((64, 128), f)], axis=0),
        "sel_b": np.concatenate(
            [np.concatenate([np.zeros((64, 64), f), np.eye(64, dtype=f)], axis=1),
             np.zeros((64, 128), f)], axis=0),
        "zeros64": np.zeros((128, 64), f),
    }


def kernel(hidden_states, attention_mask, wq, bq, wk, bk, wv, bv,
           wo, bo, ln1_g, ln1_b, wi, bi, wo2, bo2, ln2_g, ln2_b):
    nc = _get_nc()
    f = np.float32
    shared = _shared_inputs({
        "wq": wq, "wk": wk, "wv": wv, "wo": wo, "wi": wi, "wo2": wo2,
        "bq": bq, "bk": bk, "bv": bv, "bo": bo, "bi": bi, "bo2": bo2,
        "ln1_g": ln1_g, "ln1_b": ln1_b, "ln2_g": ln2_g, "ln2_b": ln2_b,
    })
    hs = np.ascontiguousarray(hidden_states, f)
    am = np.ascontiguousarray(attention_mask, f).reshape(B, S)
    in_maps = [dict(shared, x=hs[b], maskt=_perpart(am[b])) for b in range(B)]
    res = bass_utils.run_bass_kernel_spmd(nc, in_maps, core_ids=list(range(B)),
                                          trace=False)
    return np.stack([res.results[b]["out"] for b in range(B)]).astype(np.float32)



# revision 6
# speedup vs baseline: 1.4047x; 1.4047x over previous
"""Trainium2 Bass kernel for a single RoBERTa encoder layer (v2).

Problem: B=8, S=512, H=1024, 16 heads (d=64), FF=4096, fp32, eval mode.

Strategy: data-parallel over batch (one element per core, 8 cores), all
matmul operands in bf16 (host-cast, free) so FWL weight loads hide under
the matmul stream; fp32 PSUM accumulation throughout.

Per-core pipeline:
  X (bf16, host-cast) --XBAR DMA transpose--> XT (8 x [128,512] bf16)
  QT/KT = W.T @ XT (+bias via DVE tensor_scalar)  [h' on partitions]
  V normal layout -> vv tiles [tok, head-interleaved (V*e^mask | e^mask)]
    (additive attention mask folded into V''-row scaling, exact)
  per head h (64-partition contraction, no zero-padding):
    scoresT chunks [128,1024] PSUM; one Exp ACT per 2 chunks (N=1024)
    ctx accum over kpos; rows 64:128 = masked sumexp -> rinv -> ctxh
  head pairs packed via [64,128] selection matmuls
  OT = Wo.T @ ctxT; zT = (OT + bo) + XT via one scalar_tensor_tensor
  LN1 stats matmuls interleaved with WO phase; normalize -> aotT
  FFN1: gelu(Wi.T @ aotT + bi) -> interT; aotT PE-transposed back to
    normal layout (+bo2 folded) interleaved with FFN1 blocks
  FFN2 in NORMAL layout: stationary = interT slices (1 LDW / 2 MMs),
    moving = wo2 rows; accumulate [tok,1024] per s-tile; staggered f-order
    so per-s LN2 tails (sum/sumsq accum -> rstd -> fused normalize) and
    output DMAs overlap the remaining matmuls.
"""
import numpy as np
import ml_dtypes

import concourse.bass as bass
import concourse.mybir as mybir
import concourse.tile as tile
from concourse import bacc
from concourse import bass_utils
from concourse.masks import make_identity

F32 = mybir.dt.float32
BF16 = mybir.dt.bfloat16
AF = mybir.ActivationFunctionType
ALU = mybir.AluOpType

B, S, H, NH, HD, FF = 8, 512, 1024, 16, 64, 4096
KT_H = H // 128   # 8 k tiles over hidden
FT = FF // 128    # 32 f tiles over intermediate
ST = S // 128     # 4 token tiles
EPS = 1e-5

_CACHE = {}


def _build():
    nc = bacc.Bacc("TRN2", target_bir_lowering=False, debug=False,
                   enable_asserts=True, num_devices=B)

    def din(name, shape, dt=BF16):
        return nc.dram_tensor(name, shape, dt, kind="ExternalInput").ap()

    # per-core inputs
    x = din("x", [S, H], BF16)                   # host-cast bf16 hidden_states
    maskexp = din("maskexp", [128, ST], F32)     # exp(mask), per-partition layout
    # shared (host-cast bf16 weights)
    wq, wk, wv = din("wq", [H, H]), din("wk", [H, H]), din("wv", [H, H])
    wo, wi, wo2 = din("wo", [H, H]), din("wi", [H, FF]), din("wo2", [FF, H])
    bv = din("bv", [H], F32)
    bo2v = din("bo2v", [H], F32)
    g2v, b2v = din("g2v", [H], F32), din("b2v", [H], F32)
    # host-packed per-partition vectors: bq bk bo bo2 g1 b1 g2 b2 (8 cols each)
    # then bi (32 cols) -> [128, 96] fp32
    biast = din("biast", [128, 96], F32)
    ones_col = din("ones_col", [128, 1], BF16)
    ones_row = din("ones_row", [1, 128], BF16)
    ones64 = din("ones64", [128, 64], BF16)
    sel_a = din("sel_a", [64, 128], BF16)        # [I64 | 0]
    sel_b = din("sel_b", [64, 128], BF16)        # [0 | I64]
    out = nc.dram_tensor("out", [S, H], F32, kind="ExternalOutput").ap()

    from contextlib import ExitStack
    es = ExitStack()
    with tile.TileContext(nc) as tc, es:
        # LEFT stack: cst | p_xt | {qkv pools} ... then {ffn pools}
        # RIGHT stack (reverse death order): p_aotN | p_aot | p_zt | p_pair
        cst = es.enter_context(tc.tile_pool(name="cst", bufs=1))
        p_aotN = es.enter_context(tc.tile_pool(name="p_aotN", bufs=4, side="right"))
        es_aot = ExitStack()
        p_aot = es_aot.enter_context(tc.tile_pool(name="p_aot", bufs=8, side="right"))
        es_zt = ExitStack()
        p_zt = es_zt.enter_context(tc.tile_pool(name="p_zt", bufs=8, side="right"))
        es_pair = ExitStack()
        p_pair = es_pair.enter_context(
            tc.tile_pool(name="p_pair", bufs=8, side="right"))
        es_xt = ExitStack()
        p_xt = es_xt.enter_context(tc.tile_pool(name="p_xt", bufs=8))
        es_qkv = ExitStack()

        # ---- identity for PE transposes (gpsimd, early) ----
        ident = cst.tile([128, 128], BF16, tag="ident")
        make_identity(nc, ident)

        # ---- input DMAs first: x transposed via XBAR, then early weights ----
        xt = [p_xt.tile([128, S], BF16, tag="xt", name=f"xt{i}") for i in range(KT_H)]
        for k in range(KT_H):
            nc.sync.dma_start_transpose(out=xt[k], in_=x[:, 128 * k:128 * (k + 1)])

        p_w4 = es_qkv.enter_context(tc.tile_pool(name="p_w4", bufs=3))
        wqt = p_w4.tile([128, KT_H, H], BF16, tag="w4", name="wqt")
        nc.sync.dma_start(out=wqt, in_=wq.rearrange("(k p) n -> p k n", p=128))
        wkt = p_w4.tile([128, KT_H, H], BF16, tag="w4", name="wkt")
        nc.sync.dma_start(out=wkt, in_=wk.rearrange("(k p) n -> p k n", p=128))

        # ---- constants ----
        t_biast = cst.tile([128, 96], F32, tag="t_biast")
        nc.sync.dma_start(out=t_biast, in_=biast)
        t_bq, t_bk = t_biast[:, 0:8], t_biast[:, 8:16]
        t_bo = t_biast[:, 16:24]
        t_g1, t_b1 = t_biast[:, 32:40], t_biast[:, 40:48]
        t_bi = t_biast[:, 64:96]
        t_me = cst.tile([128, ST], F32, tag="t_me")
        nc.sync.dma_start(out=t_me, in_=maskexp)
        t_ones_col = cst.tile([128, 1], BF16, tag="t_ones_col")
        nc.sync.dma_start(out=t_ones_col, in_=ones_col)
        t_ones_row = cst.tile([1, 128], BF16, tag="t_ones_row")
        nc.sync.dma_start(out=t_ones_row, in_=ones_row)
        t_ones64 = cst.tile([128, 64], BF16, tag="t_ones64")
        nc.sync.dma_start(out=t_ones64, in_=ones64)
        t_sel_a = cst.tile([64, 128], BF16, tag="t_sel_a")
        nc.sync.dma_start(out=t_sel_a, in_=sel_a)
        t_sel_b = cst.tile([64, 128], BF16, tag="t_sel_b")
        nc.sync.dma_start(out=t_sel_b, in_=sel_b)
        t_bv_rep = cst.tile([128, H], F32, tag="t_bv_rep")
        nc.sync.dma_start(
            out=t_bv_rep,
            in_=bv.rearrange("(o n) -> o n", o=1).partition_broadcast(128))
        t_eps1 = cst.tile([1, 1], F32, tag="t_eps1")
        nc.vector.memset(t_eps1, EPS)
        t_eps128 = cst.tile([128, 1], F32, tag="t_eps128")
        nc.vector.memset(t_eps128, EPS)
        t_dummy = cst.tile([1, 1], F32, tag="t_dummy")
        nc.vector.memset(t_dummy, 1.0)
        t_dummy_o = cst.tile([1, 1], F32, tag="t_dummy_o")

        # warm the exp ACT table set during QKV
        nc.scalar.activation(out=t_dummy_o, in_=t_dummy, func=AF.Exp)

        # ---- phase 1: QKV projections ----
        p_qt = es_qkv.enter_context(tc.tile_pool(name="p_qt", bufs=8))
        p_kt = es_qkv.enter_context(tc.tile_pool(name="p_kt", bufs=8))
        p_v = es_qkv.enter_context(tc.tile_pool(name="p_v", bufs=4))
        qt = [p_qt.tile([128, S], BF16, tag="qt", name=f"qt{i}") for i in range(KT_H)]
        kt = [p_kt.tile([128, S], BF16, tag="kt", name=f"kt{i}") for i in range(KT_H)]
        # vv per s-tile [128, 2048]: head h cols 128h..128h+64 = V*e^mask,
        # cols 128h+64..128h+128 = e^mask (masked softmax denominator)
        vv = [p_v.tile([128, 16 * 128], BF16, tag="vv", name=f"vv{i}") for i in range(ST)]

        with tc.tile_pool(name="ps_qk", bufs=3, space="PSUM") as ps_qk, \
             tc.tile_pool(name="ps_v", bufs=2, space="PSUM") as ps_v, \
             tc.tile_pool(name="p_vt", bufs=2) as p_vt:
            for w4, bias_t, dst in ((wqt, t_bq, qt), (wkt, t_bk, kt)):
                for m in range(KT_H):
                    ps = ps_qk.tile([128, S], F32, tag="pqk")
                    for k in range(KT_H):
                        nc.tensor.matmul(ps, w4[:, k, 128 * m:128 * (m + 1)], xt[k],
                                         start=(k == 0), stop=(k == KT_H - 1))
                    nc.vector.tensor_scalar(out=dst[m], in0=ps,
                                            scalar1=bias_t[:, m:m + 1],
                                            scalar2=None, op0=ALU.add)
            # V (normal layout): stationary = xt token chunks, moving = wv rows
            wvt = p_w4.tile([128, KT_H, H], BF16, tag="w4", name="wvt")
            nc.sync.dma_start(out=wvt, in_=wv.rearrange("(k p) n -> p k n", p=128))
            for s in range(ST):
                ps = ps_v.tile([128, H], F32, tag="pv")
                for k in range(KT_H):
                    for n in range(2):
                        nc.tensor.matmul(
                            ps[:, 512 * n:512 * (n + 1)],
                            xt[k][:, 128 * s:128 * (s + 1)],
                            wvt[:, k, 512 * n:512 * (n + 1)],
                            start=(k == 0), stop=(k == KT_H - 1))
                vt = p_vt.tile([128, H], F32, tag="vt")
                nc.vector.tensor_tensor(out=vt, in0=ps, in1=t_bv_rep, op=ALU.add)
                nc.vector.tensor_scalar(
                    out=vv[s].rearrange("p (h c) -> p h c", c=128)[:, :, 0:64],
                    in0=vt.rearrange("p (h c) -> p h c", c=64),
                    scalar1=t_me[:, s:s + 1], scalar2=None, op0=ALU.mult)
                nc.vector.tensor_scalar(
                    out=vv[s].rearrange("p (h c) -> p h c", c=128)[:, :, 64:128],
                    in0=t_ones64.rearrange("p (o c) -> p o c", o=1).broadcast_to(
                        [128, 16, 64]),
                    scalar1=t_me[:, s:s + 1], scalar2=None, op0=ALU.mult)

        # prefetch wo during attention; first FFN1 chunks into cst (persist)
        wot = p_w4.tile([128, KT_H, H], BF16, tag="w4", name="wot")
        nc.sync.dma_start(out=wot, in_=wo.rearrange("(k p) n -> p k n", p=128))
        wit = {}
        for fb in range(2):
            wit[fb] = cst.tile([128, KT_H, 512], BF16, tag=f"wit{fb}",
                               name=f"wit{fb}")
            nc.sync.dma_start(
                out=wit[fb],
                in_=wi[:, 512 * fb:512 * (fb + 1)].rearrange("(k p) n -> p k n", p=128))

        # ---- phase 2: attention ----
        pair = [p_pair.tile([128, S], BF16, tag="pair", name=f"pair{i}")
                for i in range(KT_H)]
        with tc.tile_pool(name="p_expt", bufs=4) as p_expt, \
             tc.tile_pool(name="p_ctxh", bufs=4) as p_ctxh, \
             tc.tile_pool(name="p_rinv", bufs=4) as p_rinv, \
             tc.tile_pool(name="ps_sc", bufs=2, space="PSUM") as ps_sc, \
             tc.tile_pool(name="ps_ctx", bufs=3, space="PSUM") as ps_ctx:
            for t in range(KT_H):  # head pair
                ctxh = {}
                for hi, h in enumerate((2 * t, 2 * t + 1)):
                    lo, hi_ = (0, 64) if h % 2 == 0 else (64, 128)
                    qh = qt[t][lo:hi_, :]
                    kh = kt[t][lo:hi_, :]
                    es_ = []
                    for c in range(2):  # kpos chunk pairs (0,1) and (2,3)
                        ps_s = ps_sc.tile([128, 2 * S], F32, tag="sc")
                        for j in range(2):
                            kt_i = 2 * c + j
                            nc.tensor.matmul(
                                ps_s[:, 512 * j:512 * (j + 1)],
                                kh[:, 128 * kt_i:128 * (kt_i + 1)], qh,
                                start=True, stop=True)
                        e = p_expt.tile([128, 2 * S], BF16, tag="expt")
                        nc.scalar.activation(out=e, in_=ps_s, func=AF.Exp,
                                             scale=1.0 / np.sqrt(HD))
                        es_.append(e)
                    ps_c = ps_ctx.tile([128, S], F32, tag="ctx")
                    for kt_i in range(ST):
                        nc.tensor.matmul(
                            ps_c, vv[kt_i][:, 128 * h:128 * (h + 1)],
                            es_[kt_i // 2][:, 512 * (kt_i % 2):512 * (kt_i % 2 + 1)],
                            start=(kt_i == 0), stop=(kt_i == ST - 1))
                    sums_sb = p_rinv.tile([64, S], F32, tag="sums_sb")
                    nc.vector.tensor_copy(out=sums_sb, in_=ps_c[64:128, :])
                    rinv = p_rinv.tile([64, S], F32, tag="rinv")
                    nc.vector.reciprocal_approx_fast(out=rinv, in_=sums_sb)
                    ch = p_ctxh.tile([64, S], BF16, tag="ctxh")
                    nc.vector.tensor_tensor(out=ch, in0=ps_c[0:64, :], in1=rinv,
                                            op=ALU.mult)
                    ctxh[hi] = ch
                ps_p = ps_ctx.tile([128, S], F32, tag="ctx")
                nc.tensor.matmul(ps_p, t_sel_a, ctxh[0], start=True, stop=False)
                nc.tensor.matmul(ps_p, t_sel_b, ctxh[1], start=False, stop=True)
                nc.vector.tensor_copy(out=pair[t], in_=ps_p)
        # warm sqrt table for LN1 (right after last exp)
        nc.scalar.activation(out=t_dummy_o, in_=t_dummy, func=AF.Sqrt)

        # ---- phase 3: WO proj + residual + LN1 stats interleaved ----
        zt = [p_zt.tile([128, S], BF16, tag="zt", name=f"zt{i}") for i in range(KT_H)]
        aot = [p_aot.tile([128, S], BF16, tag="aot", name=f"aot{i}")
               for i in range(KT_H)]
        with tc.tile_pool(name="p_sq", bufs=3) as p_sq, \
             tc.tile_pool(name="p_stat", bufs=1) as p_stat, \
             tc.tile_pool(name="ps_wo", bufs=3, space="PSUM") as ps_wo, \
             tc.tile_pool(name="ps_stat", bufs=2, space="PSUM") as ps_stat, \
             tc.tile_pool(name="ps_rep", bufs=2, space="PSUM") as ps_rep:
            ps_sum = ps_stat.tile([1, S], F32, tag="lnsum")
            ps_sumsq = ps_stat.tile([1, S], F32, tag="lnsum")
            for m in range(KT_H):
                ps = ps_wo.tile([128, S], F32, tag="wo")
                for k in range(KT_H):
                    nc.tensor.matmul(ps, wot[:, k, 128 * m:128 * (m + 1)], pair[k],
                                     start=(k == 0), stop=(k == KT_H - 1))
                # zT = (OT + bo) + XT in one DVE op
                nc.vector.scalar_tensor_tensor(
                    out=zt[m], in0=ps, scalar=t_bo[:, m:m + 1], in1=xt[m],
                    op0=ALU.add, op1=ALU.add)
                sq = p_sq.tile([128, S], BF16, tag="sq")
                nc.vector.tensor_tensor(out=sq, in0=zt[m], in1=zt[m], op=ALU.mult)
                nc.tensor.matmul(ps_sum, t_ones_col, zt[m],
                                 start=(m == 0), stop=(m == KT_H - 1))
                nc.tensor.matmul(ps_sumsq, t_ones_col, sq,
                                 start=(m == 0), stop=(m == KT_H - 1))
            # LN1 scalar chain on [1,512]
            mu = p_stat.tile([1, S], F32, tag="mu")
            nc.vector.tensor_scalar(out=mu, in0=ps_sum, scalar1=1.0 / H,
                                    scalar2=None, op0=ALU.mult)
            ex2 = p_stat.tile([1, S], F32, tag="ex2")
            nc.vector.tensor_scalar(out=ex2, in0=ps_sumsq, scalar1=1.0 / H,
                                    scalar2=None, op0=ALU.mult)
            mu2 = p_stat.tile([1, S], F32, tag="mu2")
            nc.vector.tensor_tensor(out=mu2, in0=mu, in1=mu, op=ALU.mult)
            var = p_stat.tile([1, S], F32, tag="var")
            nc.vector.tensor_tensor(out=var, in0=ex2, in1=mu2, op=ALU.subtract)
            sd = p_stat.tile([1, S], F32, tag="sd")
            nc.scalar.activation(out=sd, in_=var, func=AF.Sqrt, bias=t_eps1,
                                 scale=1.0)
            rstd_f = p_stat.tile([1, S], F32, tag="rstd_f")
            nc.vector.reciprocal_approx_fast(out=rstd_f, in_=sd)
            mu_b = p_stat.tile([1, S], BF16, tag="mu_b")
            nc.vector.tensor_copy(out=mu_b, in_=mu)
            rstd_b = p_stat.tile([1, S], BF16, tag="rstd_b")
            nc.vector.tensor_copy(out=rstd_b, in_=rstd_f)
            ps_mu = ps_rep.tile([128, S], F32, tag="murep")
            nc.tensor.matmul(ps_mu, t_ones_row, mu_b, start=True, stop=True)
            ps_rstd = ps_rep.tile([128, S], F32, tag="murep")
            nc.tensor.matmul(ps_rstd, t_ones_row, rstd_b, start=True, stop=True)
            for m in range(KT_H):
                t1 = p_sq.tile([128, S], BF16, tag="t1")
                nc.vector.tensor_tensor(out=t1, in0=zt[m], in1=ps_mu,
                                        op=ALU.subtract)
                t2 = p_sq.tile([128, S], BF16, tag="t2")
                nc.vector.tensor_tensor(out=t2, in0=t1, in1=ps_rstd, op=ALU.mult)
                nc.scalar.activation(out=aot[m], in_=t2, func=AF.Identity,
                                     bias=t_b1[:, m:m + 1], scale=t_g1[:, m:m + 1])
        # warm gelu table for FFN1
        nc.scalar.activation(out=t_dummy_o, in_=t_dummy, func=AF.Gelu)

        # replicated LN2/bo2 constants (needed from FFN1's transposes on)
        t_bo2_rep = cst.tile([128, H], F32, tag="t_bo2_rep")
        nc.sync.dma_start(
            out=t_bo2_rep,
            in_=bo2v.rearrange("(o n) -> o n", o=1).partition_broadcast(128))
        t_g2_rep = cst.tile([128, H], F32, tag="t_g2_rep")
        nc.sync.dma_start(
            out=t_g2_rep,
            in_=g2v.rearrange("(o n) -> o n", o=1).partition_broadcast(128))
        t_b2_rep = cst.tile([128, H], F32, tag="t_b2_rep")
        nc.sync.dma_start(
            out=t_b2_rep,
            in_=b2v.rearrange("(o n) -> o n", o=1).partition_broadcast(128))

        es_pair.close()
        es_zt.close()
        es_qkv.close()
        es_xt.close()

        # ---- phase 4: FFN1 (gelu) + aot transposes to normal layout ----
        es_ffn = ExitStack()
        p_wi = es_ffn.enter_context(tc.tile_pool(name="p_wi", bufs=3))
        p_int = es_ffn.enter_context(tc.tile_pool(name="p_int", bufs=32))
        p_wo2 = es_ffn.enter_context(tc.tile_pool(name="p_wo2", bufs=4))
        intert = [p_int.tile([128, S], BF16, tag="intert", name=f"intert{i}")
                  for i in range(FT)]
        aotN = [p_aotN.tile([128, H], BF16, tag="aotN", name=f"aotN{i}")
                for i in range(ST)]
        wo2t = {}
        with tc.tile_pool(name="ps_f1", bufs=3, space="PSUM") as ps_f1, \
             tc.tile_pool(name="ps_tp", bufs=4, space="PSUM") as ps_tp:
            for fb in range(KT_H):  # 8 blocks of 4 f-tiles
                if 2 <= fb + 2 < KT_H:
                    w_n = p_wi.tile([128, KT_H, 512], BF16, tag="wi",
                                    name=f"wit{fb + 2}")
                    nc.sync.dma_start(
                        out=w_n,
                        in_=wi[:, 512 * (fb + 2):512 * (fb + 3)].rearrange(
                            "(k p) n -> p k n", p=128))
                    wit[fb + 2] = w_n
                for fi in range(4):
                    f = 4 * fb + fi
                    ps = ps_f1.tile([128, S], F32, tag="f1")
                    for k in range(KT_H):
                        nc.tensor.matmul(
                            ps, wit[fb][:, k, 128 * fi:128 * (fi + 1)], aot[k],
                            start=(k == 0), stop=(k == KT_H - 1))
                    nc.scalar.activation(out=intert[f], in_=ps, func=AF.Gelu,
                                         bias=t_bi[:, f:f + 1], scale=1.0)
                # interleave 4 residual transposes (m = fb) + wo2 prefetch
                m = fb
                for s in range(ST):
                    pt = ps_tp.tile([128, 128], BF16, tag="tp")
                    nc.tensor.transpose(pt, aot[m][:, 128 * s:128 * (s + 1)], ident)
                    nc.vector.tensor_tensor(
                        out=aotN[s][:, 128 * m:128 * (m + 1)], in0=pt,
                        in1=t_bo2_rep[:, 128 * m:128 * (m + 1)], op=ALU.add)
                if fb < 4:  # wo2 chunks 0..3 prefetched during FFN1
                    wo2t[fb] = p_wo2.tile([128, 4, H], BF16, tag="wo2",
                                          name=f"wo2t{fb}")
                    nc.sync.dma_start(
                        out=wo2t[fb],
                        in_=wo2[512 * fb:512 * (fb + 1), :].rearrange(
                            "(k p) n -> p k n", p=128))
        # warm sqrt table for LN2 (during FFN2)
        nc.scalar.activation(out=t_dummy_o, in_=t_dummy, func=AF.Sqrt)
        es_aot.close()

        # ---- phase 5: FFN2 in normal layout + LN2 + staggered stores ----
        FSPLIT = 20  # all-s interleaved up to here, then per-s finish + tail

        def ffn2_mm(f, s, ps_o):
            fc = f // 4
            if fc not in wo2t:
                wo2t[fc] = p_wo2.tile([128, 4, H], BF16, tag="wo2",
                                      name=f"wo2t{fc}")
                nc.sync.dma_start(
                    out=wo2t[fc],
                    in_=wo2[512 * fc:512 * (fc + 1), :].rearrange(
                        "(k p) n -> p k n", p=128))
            for n in range(2):
                nc.tensor.matmul(
                    ps_o[s][:, 512 * n:512 * (n + 1)],
                    intert[f][:, 128 * s:128 * (s + 1)],
                    wo2t[fc][:, f % 4, 512 * n:512 * (n + 1)],
                    start=(f == 0), stop=(f == FT - 1))

        with tc.tile_pool(name="ps_o", bufs=4, space="PSUM") as ps_op, \
             tc.tile_pool(name="p_z2", bufs=2) as p_z2, \
             tc.tile_pool(name="p_sqs", bufs=2) as p_sqs, \
             tc.tile_pool(name="p_y", bufs=2) as p_y, \
             tc.tile_pool(name="p_st2", bufs=2) as p_st2:
            ps_o = [ps_op.tile([128, H], F32, tag="o2", name=f"ps_o{s}")
                    for s in range(ST)]
            for f in range(FSPLIT):
                for s in range(ST):
                    ffn2_mm(f, s, ps_o)
            for s in range(ST):
                for f in range(FSPLIT, FT):
                    ffn2_mm(f, s, ps_o)
                # ---- LN2 tail for s ----
                z2 = p_z2.tile([128, H], F32, tag="z2")
                srow = p_st2.tile([128, 1], F32, tag="srow")
                nc.vector.scalar_tensor_tensor(
                    out=z2, in0=ps_o[s], scalar=0.0, in1=aotN[s],
                    op0=ALU.add, op1=ALU.add, accum_out=srow)
                sqs = p_sqs.tile([128, H], BF16, tag="sqs")
                sqrow = p_st2.tile([128, 1], F32, tag="sqrow")
                nc.vector.scalar_tensor_tensor(
                    out=sqs, in0=z2, scalar=0.0, in1=z2,
                    op0=ALU.add, op1=ALU.mult, accum_out=sqrow)
                mean = p_st2.tile([128, 1], F32, tag="mean")
                nc.vector.tensor_scalar(out=mean, in0=srow, scalar1=1.0 / H,
                                        scalar2=None, op0=ALU.mult)
                nm2 = p_st2.tile([128, 1], F32, tag="nm2")
                # var = sumsq/H - mean^2
                nc.vector.scalar_tensor_tensor(
                    out=nm2, in0=mean, scalar=-1.0, in1=mean,
                    op0=ALU.mult, op1=ALU.mult)
                var = p_st2.tile([128, 1], F32, tag="var")
                nc.vector.scalar_tensor_tensor(
                    out=var, in0=sqrow, scalar=1.0 / H, in1=nm2,
                    op0=ALU.mult, op1=ALU.add)
                sd2 = p_st2.tile([128, 1], F32, tag="sd2")
                nc.scalar.activation(out=sd2, in_=var, func=AF.Sqrt,
                                     bias=t_eps128, scale=1.0)
                rstd2 = p_st2.tile([128, 1], F32, tag="rstd2")
                nc.vector.reciprocal_approx_fast(out=rstd2, in_=sd2)
                ynorm = p_y.tile([128, H], F32, tag="ynorm")
                nc.vector.tensor_scalar(out=ynorm, in0=z2, scalar1=mean,
                                        scalar2=rstd2, op0=ALU.subtract,
                                        op1=ALU.mult)
                y1 = p_y.tile([128, H], F32, tag="y1")
                nc.vector.tensor_tensor(out=y1, in0=ynorm, in1=t_g2_rep,
                                        op=ALU.mult)
                y = p_y.tile([128, H], F32, tag="y", name=f"y{s}")
                nc.vector.tensor_tensor(out=y, in0=y1, in1=t_b2_rep, op=ALU.add)
                nc.sync.dma_start(out=out[128 * s:128 * (s + 1), :], in_=y)
        es_ffn.close()

    nc.compile()
    return nc


def _get_nc():
    if "nc" not in _CACHE:
        _CACHE["nc"] = _build()
    return _CACHE["nc"]


def _perpart(v):
    # [n*128] -> [128, n] with vT[p, t] = v[t*128 + p]
    v = np.asarray(v, np.float32)
    return np.ascontiguousarray(v.reshape(-1, 128).T)


def _shared_inputs(inp):
    f = np.float32
    bf = ml_dtypes.bfloat16
    biast = np.concatenate(
        [_perpart(inp["bq"]), _perpart(inp["bk"]), _perpart(inp["bo"]),
         _perpart(inp["bo2"]), _perpart(inp["ln1_g"]), _perpart(inp["ln1_b"]),
         _perpart(inp["ln2_g"]), _perpart(inp["ln2_b"]), _perpart(inp["bi"])],
        axis=1)
    cast = lambda a: np.ascontiguousarray(np.asarray(a, f).astype(bf))
    return {
        "wq": cast(inp["wq"]), "wk": cast(inp["wk"]), "wv": cast(inp["wv"]),
        "wo": cast(inp["wo"]), "wi": cast(inp["wi"]), "wo2": cast(inp["wo2"]),
        "bv": np.ascontiguousarray(inp["bv"], f),
        "bo2v": np.ascontiguousarray(inp["bo2"], f),
        "g2v": np.ascontiguousarray(inp["ln2_g"], f),
        "b2v": np.ascontiguousarray(inp["ln2_b"], f),
        "biast": biast,
        "ones_col": np.ones((128, 1), bf),
        "ones_row": np.ones((1, 128), bf),
        "ones64": np.ones((128, 64), bf),
        "sel_a": np.concatenate(
            [np.eye(64, dtype=f), np.zeros((64, 64), f)], axis=1).astype(bf),
        "sel_b": np.concatenate(
            [np.zeros((64, 64), f), np.eye(64, dtype=f)], axis=1).astype(bf),
    }


def _percore(inp, b):
    bf = ml_dtypes.bfloat16
    hs = np.asarray(inp["hidden_states"], np.float32)
    am = np.asarray(inp["attention_mask"], np.float32).reshape(B, S)
    return {
        "x": np.ascontiguousarray(hs[b].astype(bf)),
        "maskexp": _perpart(np.exp(am[b])),
    }


def kernel(hidden_states, attention_mask, wq, bq, wk, bk, wv, bv,
           wo, bo, ln1_g, ln1_b, wi, bi, wo2, bo2, ln2_g, ln2_b):
    nc = _get_nc()
    inp = {
        "hidden_states": hidden_states, "attention_mask": attention_mask,
        "wq": wq, "wk": wk, "wv": wv, "wo": wo, "wi": wi, "wo2": wo2,
        "bq": bq, "bk": bk, "bv": bv, "bo": bo, "bi": bi, "bo2": bo2,
        "ln1_g": ln1_g, "ln1_b": ln1_b, "ln2_g": ln2_g, "ln2_b": ln2_b,
    }
    shared = _shared_inputs(inp)
    in_maps = [dict(shared, **_percore(inp, b)) for b in range(B)]
    res = bass_utils.run_bass_kernel_spmd(nc, in_maps, core_ids=list(range(B)),
                                          trace=False)
    return np.stack([res.results[b]["out"] for b in range(B)]).astype(np.float32)
